# revision 8
# baseline (speedup 1.0000x reference)
"""Trainium2 Bass kernel for nn_LunaCausalAttention.

Sharding: 8 cores; core c handles batch b = c//4 and heads hs = 4*(c%4) .. hs+4.

Restructured vs baseline:
- DMA ordered so the pc projection streams against the xt tiles (no startup
  bubble); per-projection psum chains get enough banks to pipeline.
- Pass-1 computed m-major (awT = Z^T tril(G) + S^T Q), with rc folded into a
  pre-scaled qTrc at projection time, so the softmax exp emerges directly in
  the [m, tok] layout pass-2 needs -- no P~ transposes on the critical path.
- Softmax normalization deferred: P~ left unnormalized; a per-(head, token)
  scale tile (built by tiny fp16 outer-product matmuls) is applied once when
  attn^T leaves psum.
- S/T state accumulated directly in a persistent psum bank by the PE
  (start=c==0), copied to sbuf bf16 once per chunk on the Act engine.
- Output projection bias moved to the host-side partial reduction.
"""
import numpy as np

import concourse.bass as bass
import concourse.mybir as mybir
import concourse.tile as tile
from concourse import bacc
from concourse.masks import make_upper_triangular, make_identity
from concourse.bass_utils import run_bass_kernel_spmd

# static shapes
B, N, D, M, H, DH = 2, 1024, 1024, 64, 16, 64
C = 128                 # token chunk
NCH = N // C            # 8 chunks
NCORES = 8
HPC = 4                 # heads per core
E = HPC * DH            # 256 per-core head features
NF = D // 128           # 8 contraction tiles
BETA = float(np.log(2.0))
SCALE = DH ** -0.5

F32 = mybir.dt.float32
F16 = mybir.dt.float16
BF16 = mybir.dt.bfloat16
ADT = BF16              # attention-core operand dtype
AF = mybir.ActivationFunctionType
ALU = mybir.AluOpType


def build_bass():
    nc = bacc.Bacc(None, target_bir_lowering=False)

    # ---- I/O ----
    xT_d = nc.dram_tensor("xT", [D, N], BF16, kind="ExternalInput")       # query[b].T
    pT_d = nc.dram_tensor("pT", [D, M], BF16, kind="ExternalInput")       # p[b].T
    wq_d = nc.dram_tensor("wq", [D, E], BF16, kind="ExternalInput")       # scale folded
    wk_d = nc.dram_tensor("wk", [D, E], BF16, kind="ExternalInput")
    wv_d = nc.dram_tensor("wv", [D, E], BF16, kind="ExternalInput")
    wpc_d = nc.dram_tensor("wpc", [D, E], BF16, kind="ExternalInput")
    wpq_d = nc.dram_tensor("wpq", [D, E], BF16, kind="ExternalInput")     # scale folded
    wo_d = nc.dram_tensor("wo", [E, D], BF16, kind="ExternalInput")
    bq_d = nc.dram_tensor("bq", [128, 2], F32, kind="ExternalInput")      # [i,et]=b[128et+i]
    bk_d = nc.dram_tensor("bk", [128, 2], F32, kind="ExternalInput")
    bpc_d = nc.dram_tensor("bpc", [128, 2], F32, kind="ExternalInput")
    bpq_d = nc.dram_tensor("bpq", [128, 2], F32, kind="ExternalInput")
    bvr_d = nc.dram_tensor("bvr", [1, E], BF16, kind="ExternalInput")     # row form
    rcb_d = nc.dram_tensor("rcb", [128, N], F32, kind="ExternalInput")    # every row = rc
    ones_d = nc.dram_tensor("onesr", [1, 128], BF16, kind="ExternalInput")
    o2T_d = nc.dram_tensor("o2T", [2, 128], F16, kind="ExternalInput")
    ones2_d = nc.dram_tensor("ones2", [128, 2], BF16, kind="ExternalInput")  # h indicator
    out_d = nc.dram_tensor("outp", [N, D], F32, kind="ExternalOutput")

    with tile.TileContext(nc) as tc:
        with (
            tc.tile_pool(name="singles", bufs=1) as singles,
            tc.tile_pool(name="work", bufs=3) as work,
            tc.tile_pool(name="obuf", bufs=3) as obuf,
            tc.tile_pool(name="psum", bufs=1, space="PSUM") as psum,
        ):
            # ---- constants (device-generated) ----
            triu2 = singles.tile([128, 2 * C], F32)     # two upper-tri copies
            make_upper_triangular(nc, triu2[:, 0:C], val=1.0, diag=True)
            make_upper_triangular(nc, triu2[:, C:2 * C], val=1.0, diag=True)
            identb = singles.tile([128, 128], ADT)
            make_identity(nc, identb)

            # ---- DMA, in compute-need order ----
            def load_w(name, dram):
                w = singles.tile([128, NF, E], BF16, name=name)
                nc.sync.dma_start(
                    out=w, in_=dram.rearrange("(f p) e -> p f e", p=128))
                return w

            def load_small(shape, dt, dram, name):
                t = singles.tile(shape, dt, name=name)
                nc.sync.dma_start(out=t, in_=dram[:, :])
                return t

            wpc_sb = load_w("wpc_sb", wpc_d)
            bpc_sb = load_small([128, 2], F32, bpc_d, "bpc_sb")
            xt_sb = []
            for f in range(NF):
                xt = singles.tile([128, N], BF16, name=f"xt{f}")
                nc.sync.dma_start(out=xt, in_=xT_d[f * 128:(f + 1) * 128, :])
                xt_sb.append(xt)
            wk_sb = load_w("wk_sb", wk_d)
            bk_sb = load_small([128, 2], F32, bk_d, "bk_sb")
            wq_sb = load_w("wq_sb", wq_d)
            bq_sb = load_small([128, 2], F32, bq_d, "bq_sb")
            rcb_sb = singles.tile([128, N], F32)
            nc.sync.dma_start(out=rcb_sb, in_=rcb_d[:, :])
            wpq_sb = load_w("wpq_sb", wpq_d)
            pT_sb = singles.tile([128, NF, M], BF16)
            nc.sync.dma_start(
                out=pT_sb, in_=pT_d.rearrange("(f p) m -> p f m", p=128))
            bpq_sb = load_small([128, 2], F32, bpq_d, "bpq_sb")
            wv_sb = load_w("wv_sb", wv_d)
            bvr_sb = load_small([1, E], BF16, bvr_d, "bvr_sb")
            ones = load_small([1, 128], BF16, ones_d, "ones")
            o2T = load_small([2, 128], F16, o2T_d, "o2T")
            ones2 = load_small([128, 2], BF16, ones2_d, "ones2")
            wo_sb = singles.tile([128, 2, D], BF16)
            nc.sync.dma_start(
                out=wo_sb, in_=wo_d.rearrange("(t p) o -> p t o", p=128))

            # ---- persistent sbuf tiles ----
            pcT_sb = singles.tile([128, 2, N], ADT)     # [feat, hp, tok]
            kT_sb = singles.tile([128, 2, N], ADT)
            qTrc_sb = singles.tile([128, 2, N], ADT)    # q * rc(tok), bias folded
            bdpq = singles.tile([128, 2, 128], ADT)     # block-diag pq per hp
            nc.vector.memset(bdpq, 0.0)
            vtok_sb = [singles.tile([128, E], ADT, name=f"vtok{t}")
                       for t in range(NCH)]
            attnT_sb = [singles.tile([128, 2, C], ADT, name=f"attnT{t}")
                        for t in range(NCH)]
            S_sb = [singles.tile([128, M], ADT, name=f"S{hp}") for hp in range(2)]
            Tb_sb = [singles.tile([64, 128], ADT, name=f"T{hp}") for hp in range(2)]

            # persistent psum state bank:
            #   S psum: [:, 0:64] hp0, [:, 64:128] hp1  (feat-pair x m)
            #   T psum: [0:64, 128:256] hp0, [0:64, 256:384] hp1  (m x feat-pair)
            #   rowsums: [64*hp : 64*hp+2, 384:512]
            state = psum.tile([128, 512], F32, tag="state", name="state")

            # ---- pc projection, f-streamed against xt DMA ----
            for et in range(2):
                for nh in range(2):
                    pp = psum.tile([128, 512], F32, tag="pp", bufs=2, name="ppc")
                    for f in range(NF):
                        nc.tensor.matmul(
                            pp, wpc_sb[:, f, et * 128:(et + 1) * 128],
                            xt_sb[f][:, nh * 512:(nh + 1) * 512],
                            start=(f == 0), stop=(f == NF - 1))
                    nc.scalar.activation(
                        pcT_sb[:, et, nh * 512:(nh + 1) * 512], pp,
                        AF.Identity, bias=bpc_sb[:, et:et + 1])

            # ---- k projection ----
            for et in range(2):
                for nh in range(2):
                    pp = psum.tile([128, 512], F32, tag="pp", bufs=2, name="ppk")
                    for f in range(NF):
                        nc.tensor.matmul(
                            pp, wk_sb[:, f, et * 128:(et + 1) * 128],
                            xt_sb[f][:, nh * 512:(nh + 1) * 512],
                            start=(f == 0), stop=(f == NF - 1))
                    nc.scalar.activation(
                        kT_sb[:, et, nh * 512:(nh + 1) * 512], pp,
                        AF.Identity, bias=bk_sb[:, et:et + 1])

            # ---- q projection -> qTrc = (q + bq) * rc ----
            for et in range(2):
                for nh in range(2):
                    pp = psum.tile([128, 512], F32, tag="pp", bufs=2, name="ppq")
                    for f in range(NF):
                        nc.tensor.matmul(
                            pp, wq_sb[:, f, et * 128:(et + 1) * 128],
                            xt_sb[f][:, nh * 512:(nh + 1) * 512],
                            start=(f == 0), stop=(f == NF - 1))
                    nc.vector.scalar_tensor_tensor(
                        qTrc_sb[:, et, nh * 512:(nh + 1) * 512], pp,
                        bq_sb[:, et:et + 1],
                        rcb_sb[:, nh * 512:(nh + 1) * 512],
                        ALU.add, ALU.mult)

            # ---- pq projection into block-diag layout ----
            for hp in range(2):
                ppq = psum.tile([128, 512], F32, tag="pp", bufs=2, name="pppq")
                for f in range(NF):
                    nc.tensor.matmul(
                        ppq[:, 0:M], wpq_sb[:, f, hp * 128:(hp + 1) * 128],
                        pT_sb[:, f, :],
                        start=(f == 0), stop=(f == NF - 1))
                for h in range(2):
                    s = slice(64 * h, 64 * h + 64)
                    nc.vector.tensor_scalar_add(
                        bdpq[s, hp, 64 * h:64 * h + 64], ppq[s, 0:M],
                        bpq_sb[s, hp:hp + 1])

            # ---- V projection (token-major, bias via ones-row matmul) ----
            for tb in range(NCH):
                pkv = psum.tile([128, 512], F32, tag="pp", bufs=2, name="pkv")
                for f in range(NF):
                    nc.tensor.matmul(
                        pkv[:, 0:E], xt_sb[f][:, tb * 128:(tb + 1) * 128],
                        wv_sb[:, f, :],
                        start=(f == 0), stop=False)
                nc.tensor.matmul(pkv[:, 0:E], ones, bvr_sb,
                                 start=False, stop=True)
                nc.vector.tensor_copy(vtok_sb[tb], pkv[:, 0:E])

            # ---- attention ----
            def attn_call(c, hp):
                tok = slice(c * C, (c + 1) * C)
                # psum packing
                A = psum.tile([128, 512], F32, tag="pca", bufs=2, name="A")
                Bp = psum.tile([128, 512], F32, tag="pcb", bufs=2, name="Bp")
                Cp = psum.tile([128, 512], F32, tag="pcc", bufs=1, name="Cp")
                pz = A[:, 0:128]
                awT = A[:, 128:256]
                gmp = (A[:, 256:384], Bp[:, 0:128])
                g2p = (A[:, 384:512], Bp[:, 128:256])
                pan = Bp[:, 256:384]
                pkt = Cp[:, 0:64].bitcast(ADT)
                att = Cp[:, 64:128].bitcast(ADT)
                rs = Cp[64 * hp:64 * hp + 2, 128:256]

                # Z_c: pz = pcT^T @ bdpq  -> [tok, m-pair]
                nc.tensor.matmul(pz, pcT_sb[:, hp, tok], bdpq[:, hp, :],
                                 start=True, stop=True)
                ez = work.tile([128, 128], F32, name="ez")
                nc.scalar.activation(ez, pz, AF.Exp, scale=BETA)
                z = work.tile([128, 128], ADT, name="z")
                nc.scalar.activation(z, ez, AF.Ln, bias=1.0, scale=1.0)

                # K_tok via PE transpose
                ktc = work.tile([128, 128], ADT, name="ktc")
                nc.tensor.transpose(pkt, kT_sb[:, hp, tok], identb)
                nc.vector.tensor_copy(ktc, pkt)

                # Z^T via PE transpose
                at = work.tile([128, 128], ADT, name="at")
                nc.tensor.transpose(att, z, identb)
                nc.scalar.activation(at, att, AF.Copy)

                # G^T = K Q_rc^T (rc folded in qTrc), masked
                gm = work.tile([128, 256], ADT, name="gm")
                for h in range(2):
                    s = slice(64 * h, 64 * h + 64)
                    nc.tensor.matmul(gmp[h], kT_sb[s, hp, tok],
                                     qTrc_sb[s, hp, tok], start=True, stop=True,
                                     tile_position=(64 * h, 0))
                    nc.vector.tensor_mul(gm[:, 128 * h:128 * h + 128],
                                         gmp[h], triu2[:, 0:C])

                # awT[m-pair, tok] = Z^T Gm (+ S^T Q_rc)
                for h in range(2):
                    s = slice(64 * h, 64 * h + 64)
                    nc.tensor.matmul(awT[s, :], z[:, s],
                                     gm[:, 128 * h:128 * h + 128],
                                     start=True, stop=(c == 0),
                                     tile_position=(0, 64 * h))
                if c > 0:
                    for h in range(2):
                        s = slice(64 * h, 64 * h + 64)
                        nc.tensor.matmul(awT[s, :], S_sb[hp][s, :],
                                         qTrc_sb[s, hp, tok],
                                         start=False, stop=True,
                                         tile_position=(0, 64 * h))

                # P~^T = exp(awT), unnormalized, directly m-major
                pt = work.tile([128, 128], ADT, name="pt")
                nc.scalar.activation(pt, awT, AF.Exp, scale=1.0)

                # rowsums over m (partition dim) via ones2 matmul -> [2, tok]
                nc.tensor.matmul(rs, ones2, pt, start=True, stop=True,
                                 tile_position=(0, 64 * hp))
                rcp = work.tile([2, 128], F32, name="rcp")
                nc.vector.reciprocal(rcp, rs)
                rcf = work.tile([2, 128], F32, name="rcf")
                nc.vector.tensor_mul(rcf, rcp, rcb_sb[0:2, tok])
                # scale tile [feat, tok] in sbuf: partitions 64h.. = rcf row h
                scl = work.tile([128, 128], F32, name="scl")
                for h in range(2):
                    nc.gpsimd.partition_broadcast(
                        scl[64 * h:64 * h + 64, :], rcf[h:h + 1, :], channels=64)

                # pass 2: G2^T = Z P~^T, masked
                g2m = work.tile([128, 256], ADT, name="g2m")
                for h in range(2):
                    s = slice(64 * h, 64 * h + 64)
                    nc.tensor.matmul(g2p[h], at[s, :], pt[s, :],
                                     start=True, stop=True,
                                     tile_position=(0, 0))
                    nc.vector.tensor_mul(g2m[:, 128 * h:128 * h + 128],
                                         g2p[h], triu2[:, 0:C])

                # attn^T = V^T G2m (+ T^T P~^T), then normalize via scl
                for h in range(2):
                    nc.tensor.matmul(
                        pan[64 * h:64 * h + 64, :],
                        vtok_sb[c][:, hp * 128 + 64 * h:hp * 128 + 64 * h + 64],
                        g2m[:, 128 * h:128 * h + 128],
                        start=True, stop=(c == 0),
                        tile_position=(0, 64 * h))
                if c > 0:
                    for h in range(2):
                        s = slice(64 * h, 64 * h + 64)
                        nc.tensor.matmul(pan[s, :], Tb_sb[hp][:, s], pt[s, :],
                                         start=False, stop=True,
                                         tile_position=(0, 64 * h))
                nc.vector.tensor_mul(attnT_sb[c][:, hp, :], pan, scl)

                # ---- state updates (accumulate in psum, copy to sbuf) ----
                Sp = state[:, 64 * hp:64 * hp + 64]
                Tp = state[0:64, 128 + 128 * hp:256 + 128 * hp]
                for h in range(2):
                    s = slice(64 * h, 64 * h + 64)
                    nc.tensor.matmul(Sp[s, :], ktc[:, s], z[:, s],
                                     start=(c == 0), stop=True,
                                     tile_position=(0, 64 * h))
                    nc.tensor.matmul(
                        Tp[:, s], z[:, s],
                        vtok_sb[c][:, hp * 128 + 64 * h:hp * 128 + 64 * h + 64],
                        start=(c == 0), stop=True,
                        tile_position=(0, 0))
                if c < NCH - 1:
                    nc.scalar.copy(S_sb[hp], Sp)
                    nc.scalar.copy(Tb_sb[hp], Tp)

            def out_block(c):
                tok = slice(c * C, (c + 1) * C)
                for oh in range(2):
                    po = psum.tile([128, 512], F32, tag="pp", bufs=2, name="po")
                    for et in range(2):
                        nc.tensor.matmul(
                            po, attnT_sb[c][:, et, :],
                            wo_sb[:, et, oh * 512:(oh + 1) * 512],
                            start=(et == 0), stop=(et == 1))
                    ob = obuf.tile([128, 512], F32, name="ob")
                    if oh == 0:
                        nc.vector.tensor_copy(ob, po)
                    else:
                        nc.scalar.copy(ob, po)
                    nc.sync.dma_start(
                        out=out_d[tok, oh * 512:(oh + 1) * 512], in_=ob)

            for c in range(NCH):
                attn_call(c, 0)
                attn_call(c, 1)
                out_block(c)

    # Patch the act-table map so Exp and Ln both resolve to the combined
    # natural_log_exp_and_others set (otherwise the load-placement pass
    # alternates exp_and_others <-> natural_log per chunk, ~42us of reloads).
    import concourse.bacc as _bacc_mod
    from concourse.hw_specs import get_activation_tables as _gat
    _orig_gat = _bacc_mod.get_activation_tables

    def _patched_gat(arch):
        t = _gat(arch)
        for name, s in t.items():
            if name != "natural_log_exp_and_others":
                s.discard(AF.Exp)
                s.discard(AF.Ln)
        return t

    _bacc_mod.get_activation_tables = _patched_gat
    try:
        nc.compile()
    finally:
        _bacc_mod.get_activation_tables = _orig_gat
    return nc


_CACHE = {}


def _get_nc():
    if "nc" not in _CACHE:
        _CACHE["nc"] = build_bass()
    return _CACHE["nc"]


def make_in_maps(query, p, Wq, bq, Wpq, bpq, Wpc, bpc, Wk, bk, Wv, bv, Wo, bo):
    import ml_dtypes
    bf = ml_dtypes.bfloat16
    f32 = lambda a: np.ascontiguousarray(np.asarray(a), dtype=np.float32)
    query, p = f32(query), f32(p)
    Wq, Wpq, Wpc, Wk, Wv, Wo = map(f32, (Wq, Wpq, Wpc, Wk, Wv, Wo))
    bq, bpq, bpc, bk, bv, bo = map(f32, (bq, bpq, bpc, bk, bv, bo))
    rc = (1.0 / ((np.arange(N) + 1.0) * BETA)).astype(np.float32)
    rcb = np.ascontiguousarray(np.broadcast_to(rc[None, :], (128, N)))
    ones2 = np.zeros((128, 2), bf)
    ones2[0:64, 0] = 1
    ones2[64:128, 1] = 1
    o2T = np.zeros((2, 128), np.float16)
    o2T[0, 0:64] = 1
    o2T[1, 64:128] = 1

    def col2(v):  # (256,) -> (128, 2)
        return np.ascontiguousarray(v.reshape(2, 128).T)

    in_maps = []
    for core in range(NCORES):
        b = core // 4
        hs = (core % 4) * HPC
        cols = slice(hs * DH, (hs + HPC) * DH)
        m = {
            "xT": np.ascontiguousarray(query[b].T).astype(bf),
            "pT": np.ascontiguousarray(p[b].T).astype(bf),
            "wq": np.ascontiguousarray((Wq[cols, :] * SCALE).T).astype(bf),
            "wk": np.ascontiguousarray(Wk[cols, :].T).astype(bf),
            "wv": np.ascontiguousarray(Wv[cols, :].T).astype(bf),
            "wpc": np.ascontiguousarray(Wpc[cols, :].T).astype(bf),
            "wpq": np.ascontiguousarray((Wpq[cols, :] * SCALE).T).astype(bf),
            "wo": np.ascontiguousarray(Wo[:, cols].T).astype(bf),
            "bq": col2(bq[cols] * SCALE),
            "bk": col2(bk[cols]),
            "bpc": col2(bpc[cols]),
            "bpq": col2(bpq[cols] * SCALE),
            "bvr": np.ascontiguousarray(bv[cols].reshape(1, E)).astype(bf),
            "rcb": rcb,
            "onesr": np.ones((1, 128), bf),
            "o2T": o2T,
            "ones2": ones2,
        }
        in_maps.append(m)
    return in_maps


def kernel(query, p, dec_input_mask=None, p_mask=None,
           Wq=None, bq=None, Wpq=None, bpq=None, Wpc=None, bpc=None,
           Wk=None, bk=None, Wv=None, bv=None, Wo=None, bo=None,
           _trace=False, _trace_kwargs=None):
    in_maps = make_in_maps(query, p, Wq, bq, Wpq, bpq, Wpc, bpc,
                           Wk, bk, Wv, bv, Wo, bo)
    res = run_bass_kernel_spmd(_get_nc(), in_maps, core_ids=list(range(NCORES)),
                               trace=_trace, **(_trace_kwargs or {}))
    bo = np.asarray(bo, dtype=np.float32)
    out = np.zeros((B, N, D), np.float32)
    out += bo.reshape(1, 1, D)
    for core in range(NCORES):
        out[core // 4] += res.results[core]["outp"]
    if _trace:
        kernel.last_result = res
    return out
